# revision 1
# baseline (speedup 1.0000x reference)
"""Trainium2 8-core MoE layer kernel (expert-parallel, Bass/Tile).

Contract: kernel(**inputs) takes the full unsharded numpy inputs of the
MoE reference (hidden_states, router_w, w1, b1, w2, b2) and returns the
full [2, 1024, 2048] float32 output. Internally shards across 8
NeuronCores: one expert per core, replicated FFN weights in bf16,
sharded fp32 router with an AllGather of routing decisions.

Dispatch: a small f16 one-hot matmul produces, per slot of this core's
expert, the (token id, gating) pair [2, 512]; x rows are then fetched
with indirect-DMA gathers and transposed on the PE into the [H-part,
slot] layout F1 needs. Combine: expert-side weighting, indirect-DMA
scatter into a token-partitioned partial buffer, per-column-chunk
AllToAll, and an on-chip 8-way sum. An early dummy AllGather absorbs
the one-time collective-init barrier + inter-core start skew.
"""
import numpy as np
import ml_dtypes

import concourse.bass as bass
import concourse.mybir as mybir
import concourse.tile as tile

_PATCH_DOC = """Patch TileContext._drain_and_barrier: the stock version stuffs every
outstanding semaphore wait onto one SP Drain instruction; the installed
walrus rejects >1 sync wait per non-EventSemaphore instruction
("Too many sync wait commands"). Split the waits across a chain of SP
nops, then drain/barrier as before."""
import concourse.tile as tile_mod
from concourse.vector_clock import ScopedClock


def _patched_drain_and_barrier(self, tick_clock, wait_clock):
    nc = self.nc
    carrier = nc.sync.nop(nofuse=True, hint="drain_wait_carrier")
    wait_clock.add_sem_waits(
        carrier.ins, ScopedClock({None: tick_clock.global_clock})
    )
    waits = list(carrier.ins.sync_info.on_wait)
    if len(waits) > 1:
        carrier.ins.sync_info.on_wait = waits[:1]
        import bass_rust as _br
        for w in waits[1:]:
            extra = nc.sync.nop(nofuse=True, hint="drain_wait_carrier")
            extra.ins.sync_info = _br.SyncInfo(on_wait=[w], on_update=[])

    nc.sync.drain()
    nc.all_engine_barrier()
    assert self.sems is not None
    popped = nc._tile_sem_poison_stack.pop()
    assert popped is self._sem_poison
    nc.clear_and_free_semaphores(list(self.sems.allocated().values()))
    nc.all_engine_barrier()


def apply():
    tile_mod.TileContext._drain_and_barrier = _patched_drain_and_barrier


import concourse.mybir as mybir
import bass_rust as _br


def split_multi_waits(nc):
    """Walrus in this container accepts at most ONE sync wait per
    instruction. Hoist extra waits onto same-engine NoOps inserted
    immediately before the offending instruction."""
    ctr = 0
    for f in nc.m.functions:
        for b in f.blocks:
            insts = b.instructions
            need = any(
                inst.sync_info is not None and len(inst.sync_info.on_wait) > 1
                for inst in insts
            )
            if not need:
                continue
            out = []
            for inst in insts:
                si = inst.sync_info
                if si is not None and len(si.on_wait) > 1:
                    waits = list(si.on_wait)
                    for w in waits[:-1]:
                        nop = mybir.InstNoOp(name=f"I-wsplit-{ctr}", ins=[], outs=[])
                        ctr += 1
                        nop.engine = inst.engine
                        nop.sync_info = _br.SyncInfo(on_wait=[w], on_update=[])
                        out.append(nop)
                    si.on_wait = waits[-1:]
                out.append(inst)
            b.instructions = out
    return ctr


E, TOPK, CAP, H, F, N, NCORES = 8, 2, 512, 2048, 8192, 2048, 8
S = CAP
HT = H // 128                # 16 hidden tiles
FT = F // 128                # 64 ffn tiles
NQ = 8                       # combine column chunks
QH = H // NQ                 # 256
TOKC = N // NCORES           # 256
BI = N // 128                # 16
# F2 column chunks: wide (512) chunks keep LDWEIGHTS fully hidden behind
# 512-deep moving operands; the tail chunks shrink so the final
# combine + AllToAll + sum after the last matmul is short
QCHUNKS = [(0, 512), (512, 512), (1024, 512), (1536, 384), (1920, 128)]

f32 = mybir.dt.float32
f16 = mybir.dt.float16
bf16 = mybir.dt.bfloat16
i32 = mybir.dt.int32
AOP = mybir.AluOpType
AFT = mybir.ActivationFunctionType
AX = mybir.AxisListType


def build_moe(nc: bass.Bass):
    xtm = nc.dram_tensor("xtm", [2, 128, H], f32, kind="ExternalInput")
    xr = nc.dram_tensor("xr", [N, H], bf16, kind="ExternalInput")
    rwT = nc.dram_tensor("rwT", [128, HT * E], f32, kind="ExternalInput")
    w1T = nc.dram_tensor("w1tt", [FT, 128, HT * 128], bf16, kind="ExternalInput")
    w2q = nc.dram_tensor("w2q", [FT, 128, H], bf16, kind="ExternalInput")
    b1t = nc.dram_tensor("b1t", [128, FT], f32, kind="ExternalInput")
    b2r = nc.dram_tensor("b2r", [1, H], f32, kind="ExternalInput")
    cid = nc.dram_tensor("cid", [1, 1], f32, kind="ExternalInput")
    out = nc.dram_tensor("out", [TOKC, H], f32, kind="ExternalOutput")

    dmy_i = nc.dram_tensor("dmy_i", [1, 16], f32)
    dmy_o = nc.dram_tensor("dmy_o", [NCORES, 16], f32, addr_space="Shared")
    ebuf = nc.dram_tensor("ebuf", [1, 2 * N], f32)
    ebuf8 = nc.dram_tensor("ebuf8", [E, 2 * N], f32)
    e32d = nc.dram_tensor("e32d", [1, 32], f32)
    rloc = nc.dram_tensor("rloc", [TOKC, 4], f32)
    rall = nc.dram_tensor("rall", [N, 4], f32, addr_space="Shared")
    posd = nc.dram_tensor("posd", [1, 2 * N], f32)
    pq = [nc.dram_tensor(f"pq{ci}", [N, w], bf16)
          for ci, (o, w) in enumerate(QCHUNKS)]
    aq = [nc.dram_tensor(f"aq{ci}", [N, w], bf16)
          for ci, (o, w) in enumerate(QCHUNKS)]

    with tile.TileContext(nc, num_cores=NCORES) as tc:
        with tc.tile_pool(name="persist", bufs=1) as persist:
            _body(nc, tc, persist, xtm, xr, rwT, w1T, w2q, b1t, b2r, cid, out,
                  dmy_i, dmy_o, ebuf, ebuf8, e32d, rloc, rall, posd, pq, aq)
    return nc


def _body(nc, tc, persist, xtm, xr, rwT, w1T, w2q, b1t, b2r, cid, out,
          dmy_i, dmy_o, ebuf, ebuf8, e32d, rloc, rall, posd, pq, aq):
    RG = [list(range(NCORES))]

    # ---- persistent tiles ----
    b2b = persist.tile([128, H], f32, tag="b2b")
    cidb = persist.tile([128, 1], f32, tag="cidb")
    b1sb = persist.tile([128, FT], f32, tag="b1sb")
    rws = persist.tile([128, HT * E], f32, tag="rws")
    xcT = persist.tile([128, HT * S], bf16, tag="xcT")
    zbig = persist.tile([128, BI * QH], bf16, tag="zbig")
    iota512 = persist.tile([128, S], f32, tag="iota512")
    iotap = persist.tile([128, 1], f32, tag="iotap")
    ident = persist.tile([128, 128], bf16, tag="ident")
    ident2 = persist.tile([2, 2], f32, tag="ident2")
    ident8 = persist.tile([E, E], f32, tag="ident8")
    gatf = persist.tile([128, 4], f32, tag="gatf")
    desti = persist.tile([128, 4], i32, tag="desti")
    gidx = persist.tile([128, 4], i32, tag="gidx")

    nc.gpsimd.dma_start(out=b2b[:], in_=b2r[0:1, :].partition_broadcast(128).opt())
    nc.gpsimd.dma_start(out=cidb[:], in_=cid[0:1, :].partition_broadcast(128).opt())
    nc.scalar.dma_start(out=rws[:], in_=rwT[:, :])
    nc.scalar.dma_start(out=b1sb[:], in_=b1t[:, :])
    nc.vector.memset(xcT[:], 0.0)
    nc.vector.memset(zbig[:], 0.0)
    with tc.tile_pool(name="iota_tmp", bufs=1) as it_p:
        ii = it_p.tile([128, S], i32, tag="ii")
        nc.gpsimd.iota(ii[:], pattern=[[1, S]], base=0, channel_multiplier=0)
        nc.vector.tensor_copy(out=iota512[:], in_=ii[:])
        ip = it_p.tile([128, 1], i32, tag="ip")
        nc.gpsimd.iota(ip[:], pattern=[[0, 1]], base=0, channel_multiplier=1)
        nc.vector.tensor_copy(out=iotap[:], in_=ip[:])
        nc.vector.tensor_scalar(out=ident[:], in0=iota512[:, 0:128],
                                scalar1=iotap[:], scalar2=None, op0=AOP.is_equal)
        i2 = it_p.tile([2, 2], i32, tag="i2")
        nc.gpsimd.iota(i2[:], pattern=[[1, 2]], base=0, channel_multiplier=0)
        i2f = it_p.tile([2, 2], f32, tag="i2f")
        nc.vector.tensor_copy(out=i2f[:], in_=i2[:])
        ip2 = it_p.tile([2, 1], i32, tag="ip2")
        nc.gpsimd.iota(ip2[:], pattern=[[0, 1]], base=0, channel_multiplier=1)
        ip2f = it_p.tile([2, 1], f32, tag="ip2f")
        nc.vector.tensor_copy(out=ip2f[:], in_=ip2[:])
        nc.vector.tensor_scalar(out=ident2[:], in0=i2f[:], scalar1=ip2f[:],
                                scalar2=None, op0=AOP.is_equal)
        i8 = it_p.tile([E, E], i32, tag="i8")
        nc.gpsimd.iota(i8[:], pattern=[[1, E]], base=0, channel_multiplier=0)
        i8f = it_p.tile([E, E], f32, tag="i8f")
        nc.vector.tensor_copy(out=i8f[:], in_=i8[:])
        ip8 = it_p.tile([E, 1], i32, tag="ip8")
        nc.gpsimd.iota(ip8[:], pattern=[[0, 1]], base=0, channel_multiplier=1)
        ip8f = it_p.tile([E, 1], f32, tag="ip8f")
        nc.vector.tensor_copy(out=ip8f[:], in_=ip8[:])
        nc.vector.tensor_scalar(out=ident8[:], in0=i8f[:], scalar1=ip8f[:],
                                scalar2=None, op0=AOP.is_equal)

    # ============ Phase R: sharded router (own 256 tokens, fp32) ============
    with (tc.tile_pool(name="r_x", bufs=2) as r_x,
          tc.tile_pool(name="r_ps", bufs=2, space="PSUM") as r_ps,
          tc.tile_pool(name="r_sb", bufs=2) as r_sb):
        for tt2 in range(2):
            xt_t = r_x.tile([128, H], f32, tag="xt_t")
            for qq in range(2):
                eng = nc.sync if qq == 0 else nc.scalar
                eng.dma_start(
                    out=xt_t[:, qq * (H // 2):(qq + 1) * (H // 2)],
                    in_=xtm[tt2, :, qq * (H // 2):(qq + 1) * (H // 2)])
            ps = r_ps.tile([128, E], f32, tag="r_ps")
            for hc in range(HT):
                nc.tensor.matmul(
                    out=ps[:], lhsT=xt_t[:, hc * 128:(hc + 1) * 128],
                    rhs=rws[:, hc * E:(hc + 1) * E],
                    start=(hc == 0), stop=(hc == HT - 1))
            lsb = r_sb.tile([128, E], f32, tag="lsb")
            nc.vector.tensor_copy(out=lsb[:], in_=ps[:])
            mx = r_sb.tile([128, 1], f32, tag="mx")
            nc.vector.tensor_reduce(out=mx[:], in_=lsb[:], op=AOP.max, axis=AX.X)
            nm = r_sb.tile([128, 1], f32, tag="nm")
            nc.vector.tensor_scalar_mul(nm[:], mx[:], -1.0)
            ex = r_sb.tile([128, E], f32, tag="ex")
            ssum = r_sb.tile([128, 1], f32, tag="ssum")
            nc.scalar.activation(out=ex[:], in_=lsb[:], func=AFT.Exp,
                                 bias=nm[:], scale=1.0, accum_out=ssum[:])
            rcp = r_sb.tile([128, 1], f32, tag="rcp")
            nc.vector.reciprocal(out=rcp[:], in_=ssum[:])
            pr = r_sb.tile([128, E], f32, tag="pr")
            nc.vector.tensor_scalar_mul(pr[:], ex[:], rcp[:])
            mx8 = r_sb.tile([128, 8], f32, tag="mx8")
            ix8 = r_sb.tile([128, 8], mybir.dt.uint32, tag="ix8")
            nc.vector.max_with_indices(out_max=mx8[:], out_indices=ix8[:],
                                       in_=pr[:])
            rv = r_sb.tile([128, 4], f32, tag="rv")
            nc.vector.tensor_copy(out=rv[:, 0:1], in_=ix8[:, 0:1])
            nc.vector.tensor_copy(out=rv[:, 1:2], in_=ix8[:, 1:2])
            nc.vector.tensor_copy(out=rv[:, 2:3], in_=mx8[:, 0:1])
            nc.vector.tensor_copy(out=rv[:, 3:4], in_=mx8[:, 1:2])
            nc.scalar.dma_start(out=rloc[tt2 * 128:(tt2 + 1) * 128, :], in_=rv[:])
        nc.gpsimd.collective_compute(
            "AllGather", AOP.bypass,
            replica_groups=RG,
            ins=[rloc[:, :].opt()],
            outs=[rall[:, :].opt()])

    # zero the combine partial buffers (fresh every run); runs on the idle
    # gpsimd queue during the AllGather wait
    for ci, (off, w) in enumerate(QCHUNKS):
        rows = 16
        while rows * w > 4096:
            rows //= 2
        nper = N // (rows * 128)
        for z in range(nper):
            nc.gpsimd.dma_start(
                out=pq[ci][z * rows * 128:(z + 1) * rows * 128, :].rearrange(
                    "(c p) f -> p c f", p=128),
                in_=zbig[:, :rows * w].rearrange("p (c f) -> p c f", f=w))

    # contiguous per-partition load of the gathered routing, (p, b) layout
    rb = persist.tile([128, 16 * 4], f32, tag="rb")
    nc.scalar.dma_start(
        out=rb[:], in_=rall[:, :].rearrange("(p b) c -> p (b c)", p=128))
    rbv = rb[:].rearrange("p (b c) -> p b c", c=4)
    e0a = persist.tile([128, BI], f32, tag="e0a")
    e1a = persist.tile([128, BI], f32, tag="e1a")
    p0a = persist.tile([128, BI], f32, tag="p0a")
    p1a = persist.tile([128, BI], f32, tag="p1a")
    nc.vector.tensor_copy(out=e0a[:], in_=rbv[:, :, 0].opt())
    nc.vector.tensor_copy(out=e1a[:], in_=rbv[:, :, 1].opt())
    nc.vector.tensor_copy(out=p0a[:], in_=rbv[:, :, 2].opt())
    nc.vector.tensor_copy(out=p1a[:], in_=rbv[:, :, 3].opt())

    # ============ Phase S: one-hot + 4-way segmented scan (fp16) ============
    # pack expert ids, roundtrip through DRAM to get the (k, t)-ordered
    # row (t = p*16 + b), then a broadcast load into [32, 1024]: partition
    # (e, seg) scans its 1024-long segment; segment offsets fixed up via a
    # small triangular matmul over the per-segment totals.
    NSEG = 4
    SEGL = 2 * N // NSEG
    with (tc.tile_pool(name="scan", bufs=1) as sc,
          tc.tile_pool(name="s_ps", bufs=2, space="PSUM") as s_ps):
        ip32 = sc.tile([32, 1], i32, tag="ip32")
        ip32f = sc.tile([32, 1], f32, tag="ip32f")
        nc.gpsimd.iota(ip32[:], pattern=[[0, 1]], base=0, channel_multiplier=1)
        nc.vector.tensor_copy(out=ip32f[:], in_=ip32[:])
        eri = sc.tile([1, 32], i32, tag="eri")
        nc.gpsimd.iota(eri[:], pattern=[[1, E], [0, NSEG]], base=0,
                       channel_multiplier=0)
        erf = sc.tile([1, 32], f32, tag="erf")
        nc.vector.tensor_copy(out=erf[:], in_=eri[:])
        nc.scalar.dma_start(out=e32d[0:1, :], in_=erf[:])
        eidx = sc.tile([32, 1], f32, tag="eidx")
        nc.scalar.dma_start(
            out=eidx[:, :],
            in_=e32d[0:1, :].rearrange("a (c u) -> (a c) u", u=1))
        sidx = sc.tile([32, 1], f32, tag="sidx")
        nc.vector.scalar_tensor_tensor(out=sidx[:], in0=eidx[:],
                                       scalar=-float(NSEG), in1=ip32f[:],
                                       op0=AOP.mult, op1=AOP.add)
        # Mt[p', p] = same expert and seg(p') < seg(p): exclusive prefix mask
        jmi = sc.tile([32, 32], i32, tag="jmi")
        nc.gpsimd.iota(jmi[:], pattern=[[0, E], [1, NSEG]], base=0,
                       channel_multiplier=0)
        jm = sc.tile([32, 32], f32, tag="jm")
        nc.vector.tensor_copy(out=jm[:], in_=jmi[:])
        eci = sc.tile([32, 32], i32, tag="eci")
        nc.gpsimd.iota(eci[:], pattern=[[1, E], [0, NSEG]], base=0,
                       channel_multiplier=0)
        ec = sc.tile([32, 32], f32, tag="ec")
        nc.vector.tensor_copy(out=ec[:], in_=eci[:])
        Mt = sc.tile([32, 32], f16, tag="Mt")
        me32 = sc.tile([32, 32], f32, tag="me32")
        nc.vector.tensor_scalar(out=me32[:], in0=ec[:],
                                scalar1=eidx[:], scalar2=None, op0=AOP.is_equal)
        ms32 = sc.tile([32, 32], f32, tag="ms32")
        nc.vector.tensor_scalar(out=ms32[:], in0=jm[:],
                                scalar1=sidx[:], scalar2=None, op0=AOP.is_gt)
        nc.vector.tensor_tensor(out=Mt[:], in0=me32[:], in1=ms32[:],
                                op=AOP.mult)
        sel4 = sc.tile([32, NSEG], f16, tag="sel4")
        iseg = sc.tile([32, NSEG], i32, tag="iseg")
        nc.gpsimd.iota(iseg[:], pattern=[[1, NSEG]], base=0, channel_multiplier=0)
        isegf = sc.tile([32, NSEG], f32, tag="isegf")
        nc.vector.tensor_copy(out=isegf[:], in_=iseg[:])
        nc.vector.tensor_scalar(out=sel4[:], in0=isegf[:], scalar1=sidx[:],
                                scalar2=None, op0=AOP.is_equal)

        e01 = sc.tile([128, 32], f32, tag="e01")
        nc.vector.tensor_copy(out=e01[:, 0:16], in_=e0a[:])
        nc.vector.tensor_copy(out=e01[:, 16:32], in_=e1a[:])
        for e in range(E):
            eng = nc.scalar if e % 2 == 0 else nc.sync
            eng.dma_start(
                out=ebuf8[e:e + 1, :].rearrange(
                    "a (k p b) -> (a p) k b", k=2, p=128),
                in_=e01[:].rearrange("p (k b) -> p k b", k=2))
        ohsrc = sc.tile([32, SEGL], f32, tag="ohsrc")
        nc.scalar.dma_start(
            out=ohsrc[:],
            in_=ebuf8[:, :].rearrange("e (s c) -> (e s) c", s=NSEG))
        ohcat = sc.tile([32, SEGL], f16, tag="ohcat")
        nc.vector.tensor_scalar(out=ohcat[:], in0=ohsrc[:], scalar1=eidx[:],
                                scalar2=None, op0=AOP.is_equal)
        ones2n = sc.tile([32, SEGL], f16, tag="ones2n")
        nc.vector.memset(ones2n[:], 1.0)
        cum = sc.tile([32, SEGL], f16, tag="cum")
        nc.vector.tensor_tensor_scan(out=cum[:], data0=ones2n[:], data1=ohcat[:],
                                     initial=0.0, op0=AOP.mult, op1=AOP.add)
        tot32 = sc.tile([32, 1], f16, tag="tot32")
        with nc.allow_low_precision(reason="segment counts <= 536, f16-exact"):
            nc.vector.tensor_reduce(out=tot32[:], in_=ohcat[:], op=AOP.add,
                                    axis=AX.X)
        offp = s_ps.tile([32, 1], f32, tag="offp")
        nc.tensor.matmul(out=offp[:], lhsT=Mt[:], rhs=tot32[:],
                         start=True, stop=True)
        off32 = sc.tile([32, 1], f32, tag="off32")
        nc.vector.tensor_copy(out=off32[:], in_=offp[:])
        cumf = sc.tile([32, SEGL], f16, tag="cumf")
        nc.vector.tensor_scalar(out=cumf[:], in0=cum[:], scalar1=off32[:],
                                scalar2=None, op0=AOP.add)
        ohcum = sc.tile([32, SEGL], f16, tag="ohcum")
        nc.vector.tensor_tensor(out=ohcum[:], in0=ohcat[:], in1=cumf[:],
                                op=AOP.mult)
        posrow = sc.tile([1, 2 * N], f32, tag="posrow")
        for s in range(NSEG):
            for ch in range(SEGL // 512):
                pps = s_ps.tile([1, 512], f32, tag="pps")
                nc.tensor.matmul(out=pps[:], lhsT=sel4[:, s:s + 1],
                                 rhs=ohcum[:, ch * 512:(ch + 1) * 512],
                                 start=True, stop=True)
                nc.vector.tensor_scalar_add(
                    posrow[:, s * SEGL + ch * 512:s * SEGL + (ch + 1) * 512],
                    pps[:], -1.0)
        nc.scalar.dma_start(out=posd[:, 0:N], in_=posrow[:, 0:N])
        nc.scalar.dma_start(out=posd[:, N:2 * N], in_=posrow[:, N:2 * N])

    # ============ Phase I: slot table via one-hot matmul ============
    # token t lives at [partition t%128, col t//128]; pair tiles j = k*16 + c.
    with (tc.tile_pool(name="imath", bufs=1) as im,
          tc.tile_pool(name="oh_ps", bufs=2, space="PSUM") as oh_ps):
        pos0a = im.tile([128, BI], f32, tag="pos0a")
        pos1a = im.tile([128, BI], f32, tag="pos1a")
        nc.scalar.dma_start(
            out=pos0a[:], in_=posd[0:1, 0:N].rearrange("a (p b) -> (a p) b", p=128))
        nc.sync.dma_start(
            out=pos1a[:],
            in_=posd[0:1, N:2 * N].rearrange("a (p b) -> (a p) b", p=128))
        ka_ps = oh_ps.tile([1, BI], f32, tag="ka_ps")
        nc.tensor.matmul(out=ka_ps[:], lhsT=iotap[:], rhs=pos0a[:],
                         start=True, stop=True)
        ka_ps2 = oh_ps.tile([1, BI], f32, tag="ka_ps")
        nc.tensor.matmul(out=ka_ps2[:], lhsT=iotap[:], rhs=pos1a[:],
                         start=True, stop=True)
        # masked slot positions: ps_k = pos_k if (e_k == mine and kept) else -1
        psm = im.tile([128, 2 * BI], f32, tag="psm")
        for k, (ea, pa) in enumerate([(e0a, pos0a), (e1a, pos1a)]):
            mk = im.tile([128, BI], f32, tag=f"mk{k}")
            nc.vector.tensor_scalar(out=mk[:], in0=pa[:], scalar1=float(CAP),
                                    scalar2=None, op0=AOP.is_lt)
            me = im.tile([128, BI], f32, tag=f"me{k}")
            nc.vector.tensor_scalar(out=me[:], in0=ea[:], scalar1=cidb[:],
                                    scalar2=None, op0=AOP.is_equal)
            nc.vector.tensor_tensor(out=mk[:], in0=mk[:], in1=me[:], op=AOP.mult)
            pp1 = im.tile([128, BI], f32, tag=f"pp1{k}")
            nc.vector.scalar_tensor_tensor(out=pp1[:], in0=pa[:], scalar=1.0,
                                           in1=mk[:], op0=AOP.add, op1=AOP.mult)
            nc.vector.tensor_scalar_add(psm[:, k * BI:(k + 1) * BI], pp1[:], -1.0)
        ka_ps3 = oh_ps.tile([1, BI], f32, tag="ka_ps")
        nc.tensor.matmul(out=ka_ps3[:], lhsT=iotap[:], rhs=psm[:, 0:BI],
                         start=True, stop=True)
        ka_ps4 = oh_ps.tile([1, BI], f32, tag="ka_ps")
        nc.tensor.matmul(out=ka_ps4[:], lhsT=iotap[:], rhs=psm[:, BI:2 * BI],
                         start=True, stop=True)
        # lhsT table L: per pair tile j, cols [2j, 2j+1] = (token id, gating)
        tokm = im.tile([128, BI], i32, tag="tokm")
        nc.gpsimd.iota(tokm[:], pattern=[[1, BI]], base=0, channel_multiplier=BI)
        L = im.tile([128, 64], f16, tag="L")
        Lv = L[:].rearrange("p (j c) -> p j c", c=2)
        nc.vector.tensor_copy(out=Lv[:, 0:16, 0].opt(), in_=tokm[:])
        nc.vector.tensor_copy(out=Lv[:, 16:32, 0].opt(), in_=tokm[:])
        nc.vector.tensor_copy(out=Lv[:, 0:16, 1].opt(), in_=p0a[:])
        nc.vector.tensor_copy(out=Lv[:, 16:32, 1].opt(), in_=p1a[:])
        # one-hot tiles and accumulation into PT [2, 512]
        pt_ps = oh_ps.tile([2, S], f32, tag="pt_ps")
        for j in range(32):
            ohj = im.tile([128, S], f16, tag=f"oh{j}")
            nc.vector.tensor_scalar(out=ohj[:], in0=iota512[:],
                                    scalar1=psm[:, j:j + 1], scalar2=None,
                                    op0=AOP.is_equal)
            nc.tensor.matmul(out=pt_ps[:], lhsT=L[:, 2 * j:2 * j + 2],
                             rhs=ohj[:], start=(j == 0), stop=(j == 31))
        PTs = im.tile([2, S], f32, tag="PTs")
        nc.vector.tensor_copy(out=PTs[:], in_=pt_ps[:])
        # transpose [2, 512] -> [128, 4, 2]; emit each gidx column as soon
        # as its chunk transposes so the x-row gathers start immediately
        IG = im.tile([128, 4 * 2], f32, tag="IG")
        IGv = IG[:].rearrange("p (c t) -> p c t", t=2)
        for c in range(4):
            tps = oh_ps.tile([128, 2], f32, tag="tpc")
            nc.tensor.transpose(out=tps[:], in_=PTs[:, c * 128:(c + 1) * 128],
                                identity=ident2[:])
            nc.vector.tensor_copy(out=IGv[:, c, :].opt(), in_=tps[:])
            nc.vector.tensor_copy(out=gidx[:, c:c + 1], in_=IGv[:, c, 0:1].opt())
        tokf = im.tile([128, 4], f32, tag="tokf")
        nc.vector.tensor_copy(out=tokf[:], in_=IGv[:, :, 0].opt())
        nc.vector.tensor_copy(out=gatf[:], in_=IGv[:, :, 1].opt())
        # pad slots (gating == 0) -> out-of-range dest, dropped by scatter
        mz = im.tile([128, 4], f32, tag="mz")
        nc.vector.tensor_scalar(out=mz[:], in0=gatf[:], scalar1=0.0,
                                scalar2=None, op0=AOP.is_equal)
        dstf = im.tile([128, 4], f32, tag="dstf")
        nc.vector.scalar_tensor_tensor(out=dstf[:], in0=mz[:], scalar=8192.0,
                                       in1=tokf[:], op0=AOP.mult, op1=AOP.add)
        nc.vector.tensor_copy(out=desti[:], in_=dstf[:])

    # ============ Phase D: gather x rows + PE transpose into xcT ============
    with (tc.tile_pool(name="xg", bufs=4) as xg_pool,
          tc.tile_pool(name="tr_ps", bufs=4, space="PSUM") as tr_ps):
        xgs = []
        for st in range(4):
            xg = xg_pool.tile([128, H], bf16, tag=f"xg{st}")
            nc.gpsimd.indirect_dma_start(
                out=xg[:], out_offset=None,
                in_=xr[:, :],
                in_offset=bass.IndirectOffsetOnAxis(ap=gidx[:, st:st + 1], axis=0))
            xgs.append(xg)
        # hc-major so F1 can start on early h-tiles while later ones transpose
        for hc in range(HT):
            for st in range(4):
                tp = tr_ps.tile([128, 128], bf16, tag="tp")
                nc.tensor.transpose(
                    out=tp[:], in_=xgs[st][:, hc * 128:(hc + 1) * 128],
                    identity=ident[:])
                eng = nc.scalar if st % 2 == 0 else nc.vector
                if eng is nc.scalar:
                    eng.copy(out=xcT[:, hc * S + st * 128:hc * S + (st + 1) * 128],
                             in_=tp[:])
                else:
                    eng.tensor_copy(
                        out=xcT[:, hc * S + st * 128:hc * S + (st + 1) * 128],
                        in_=tp[:])

    # ============ Phase F1 ============
    with tc.tile_pool(name="g", bufs=1) as g_pool:
        g = []
        with (tc.tile_pool(name="f1_w", bufs=4) as f1_w,
              tc.tile_pool(name="f1_ps", bufs=2, space="PSUM") as f1_ps):
            for ft in range(FT):
                w1_t = f1_w.tile([128, HT * 128], bf16, tag="w1_t")
                QW = HT * 128 // 2
                for qq in range(2):
                    eng = nc.sync if qq == 0 else nc.scalar
                    eng.dma_start(
                        out=w1_t[:, qq * QW:(qq + 1) * QW],
                        in_=w1T[ft, :, qq * QW:(qq + 1) * QW])
                ps = f1_ps.tile([128, S], f32, tag="f1_ps")
                for hc in range(HT):
                    nc.tensor.matmul(
                        out=ps[:], lhsT=w1_t[:, hc * 128:(hc + 1) * 128],
                        rhs=xcT[:, hc * S:(hc + 1) * S],
                        start=(hc == 0), stop=(hc == HT - 1))
                gt = g_pool.tile([128, S], bf16, tag=f"g_{ft}")
                nc.scalar.activation(out=gt[:], in_=ps[:], func=AFT.Gelu,
                                     bias=b1sb[:, ft:ft + 1], scale=1.0)
                g.append(gt)

        # ============ Phase F2 + combine + AllToAll per column chunk ====
        # consume (t8 load + 8-way sum + store) runs one chunk behind so
        # the gpsimd queue never blocks on an in-flight AllToAll
        with (tc.tile_pool(name="f2_w", bufs=18) as f2_w,
              tc.tile_pool(name="f2_ps", bufs=2, space="PSUM") as f2_ps,
              tc.tile_pool(name="cbp", bufs=2) as cbp,
              tc.tile_pool(name="smp", bufs=2) as smp):

            def consume(ci, off, w):
                wt = f"w{w}"
                for tt2 in range(2):
                    t8 = smp.tile([128, NCORES * w], bf16, tag=f"t8{wt}")
                    nc.gpsimd.dma_start(
                        out=t8[:].rearrange("p (s f) -> p s f", s=NCORES),
                        in_=aq[ci][:, :].rearrange(
                            "(s u p) f -> u p s f", u=2, p=128)[tt2].opt())
                    acc = smp.tile([128, w], f32, tag=f"acc{wt}")
                    nc.vector.tensor_tensor(
                        out=acc[:], in0=t8[:, 0:w], in1=t8[:, w:2 * w],
                        op=AOP.add)
                    for s in range(2, NCORES):
                        nc.vector.tensor_tensor(
                            out=acc[:], in0=acc[:],
                            in1=t8[:, s * w:(s + 1) * w], op=AOP.add)
                    nc.gpsimd.dma_start(
                        out=out[tt2 * 128:(tt2 + 1) * 128, off:off + w],
                        in_=acc[:])

            for ci, (off, w) in enumerate(QCHUNKS):
                wt = f"w{w}"
                psq = []
                for mt in range(4):
                    psq_t = f2_ps.tile([128, 512], f32, tag=f"f2_ps_{mt}")
                    psq.append(psq_t)
                for fc in range(FT):
                    w2_t = f2_w.tile([128, w], bf16, tag=f"w2_t{wt}")
                    eng = nc.sync if (fc % 2 == 0) else nc.scalar
                    eng.dma_start(out=w2_t[:], in_=w2q[fc, :, off:off + w])
                    for mt in range(4):
                        nc.tensor.matmul(
                            out=psq[mt][:, :w],
                            lhsT=g[fc][:, mt * 128:(mt + 1) * 128],
                            rhs=w2_t[:],
                            start=(fc == 0), stop=(fc == FT - 1))
                if ci > 1:
                    consume(ci - 2, *QCHUNKS[ci - 2])
                for mt in range(4):
                    hs = cbp.tile([128, w], f32, tag=f"hs{wt}")
                    nc.vector.tensor_tensor(
                        out=hs[:], in0=psq[mt][:, :w],
                        in1=b2b[:, off:off + w], op=AOP.add)
                    comb = cbp.tile([128, w], bf16, tag=f"comb{wt}")
                    nc.vector.tensor_scalar_mul(
                        comb[:], hs[:], gatf[:, mt:mt + 1])
                    nc.gpsimd.indirect_dma_start(
                        out=pq[ci][:, :], in_=comb[:],
                        out_offset=bass.IndirectOffsetOnAxis(
                            ap=desti[:, mt:mt + 1], axis=0),
                        in_offset=None,
                        bounds_check=N - 1, oob_is_err=False)
                nc.gpsimd.collective_compute(
                    "AllToAll", AOP.bypass, replica_groups=RG,
                    ins=[pq[ci][:, :].opt()],
                    outs=[aq[ci][:, :].opt()])
            consume(len(QCHUNKS) - 2, *QCHUNKS[-2])
            consume(len(QCHUNKS) - 1, *QCHUNKS[-1])


# ======================== host-side glue ========================

_CACHE = {}


def _prep_inputs(hidden_states, router_w, w1, b1, w2, b2):
    x = np.asarray(hidden_states, np.float32).reshape(-1, H)
    xT = x.T
    xr = x.astype(ml_dtypes.bfloat16)
    w1Tm = np.asarray(w1, np.float32).T.astype(ml_dtypes.bfloat16)
    w2Tm = np.asarray(w2, np.float32).T.astype(ml_dtypes.bfloat16)
    w1tt = np.ascontiguousarray(
        w1Tm.reshape(16, 128, 64, 128).transpose(2, 1, 0, 3)).reshape(64, 128, 2048)
    w2qm = np.ascontiguousarray(w2Tm.reshape(64, 128, 2048))
    base = {
        "xr": np.ascontiguousarray(xr),
        "w1tt": w1tt,
        "w2q": w2qm,
        "rwT": np.ascontiguousarray(
            np.asarray(router_w, np.float32).T.reshape(16, 128, 8)
            .transpose(1, 0, 2).reshape(128, 128)),
        "b1t": np.ascontiguousarray(np.asarray(b1, np.float32).reshape(FT, 128).T),
        "b2r": np.asarray(b2, np.float32).reshape(1, H),
    }
    xtmf = np.ascontiguousarray(
        xT.reshape(16, 128, 16, 128).transpose(2, 1, 0, 3)).reshape(16, 128, 2048)
    ins = []
    for c in range(NCORES):
        m = dict(base)
        m["xtm"] = np.ascontiguousarray(xtmf[2 * c:2 * c + 2])
        m["cid"] = np.full((1, 1), float(c), np.float32)
        ins.append(m)
    return ins


def _get_nc():
    if "nc" not in _CACHE:
        apply()  # tile drain patch
        nc = bass.Bass(num_devices=NCORES)
        build_moe(nc)
        split_multi_waits(nc)
        _CACHE["nc"] = nc
    return _CACHE["nc"]


def kernel(hidden_states, router_w, w1, b1, w2, b2):
    from concourse.bass_utils import run_bass_kernel_spmd

    orig_shape = np.asarray(hidden_states).shape
    nc = _get_nc()
    ins = _prep_inputs(hidden_states, router_w, w1, b1, w2, b2)
    res = run_bass_kernel_spmd(nc, ins, core_ids=list(range(NCORES)))
    full = np.concatenate([res.results[c]["out"] for c in range(NCORES)], axis=0)
    return full.reshape(orig_shape).astype(np.float32)



# revision 4
# speedup vs baseline: 2.2024x; 2.2024x over previous
"""Trainium2 8-core MoE layer kernel (token-parallel dense FFN, Bass/Tile).

Contract: kernel(**inputs) takes the full unsharded numpy inputs of the
MoE reference (hidden_states, router_w, w1, b1, w2, b2) and returns the
full [2, 1024, 2048] float32 output.

Key identity: the reference's experts all share one FFN (w1/b1/w2/b2 are
not per-expert), so for every kept (token, k) slot the expert output is
FFN(x[t]) and the combine collapses to
    out[t] = (sum_k kept_k(t) * gate_k(t)) * FFN(x[t]).
Routing therefore only determines a per-token scalar; the FFN itself is
a dense [N, H] pass, token-sharded 8 ways (256 tokens per core, half the
dispatch-buffer FLOPs, no gather/scatter, no AllToAll).

Per core: fp32 router on own 256 tokens -> AllGather of the [N, 4]
routing decisions -> replicated capacity scan (positions of all 2N
routed slots) -> per-token weight w(t) -> dense F1 (gelu) + F2 with
streamed bf16 weights -> scale by w(t), write own output shard. The
routing -> w(t) chain runs on vector/gpsimd during F1; its two tiny
matmul groups issue between F1 and F2 on the tensor queue.
"""
import numpy as np
import ml_dtypes

import concourse.bass as bass
import concourse.mybir as mybir
import concourse.tile as tile

_PATCH_DOC = """Patch TileContext._drain_and_barrier: the stock version stuffs every
outstanding semaphore wait onto one SP Drain instruction; the installed
walrus rejects >1 sync wait per non-EventSemaphore instruction
("Too many sync wait commands"). Split the waits across a chain of SP
nops, then drain/barrier as before."""
import concourse.tile as tile_mod
from concourse.vector_clock import ScopedClock


def _patched_drain_and_barrier(self, tick_clock, wait_clock):
    nc = self.nc
    carrier = nc.sync.nop(nofuse=True, hint="drain_wait_carrier")
    wait_clock.add_sem_waits(
        carrier.ins, ScopedClock({None: tick_clock.global_clock})
    )
    waits = list(carrier.ins.sync_info.on_wait)
    if len(waits) > 1:
        carrier.ins.sync_info.on_wait = waits[:1]
        import bass_rust as _br
        for w in waits[1:]:
            extra = nc.sync.nop(nofuse=True, hint="drain_wait_carrier")
            extra.ins.sync_info = _br.SyncInfo(on_wait=[w], on_update=[])

    nc.sync.drain()
    nc.all_engine_barrier()
    assert self.sems is not None
    popped = nc._tile_sem_poison_stack.pop()
    assert popped is self._sem_poison
    nc.clear_and_free_semaphores(list(self.sems.allocated().values()))
    nc.all_engine_barrier()


def apply():
    tile_mod.TileContext._drain_and_barrier = _patched_drain_and_barrier


import concourse.mybir as mybir
import bass_rust as _br


def split_multi_waits(nc):
    """Walrus in this container accepts at most ONE sync wait per
    instruction. Hoist extra waits onto same-engine NoOps inserted
    immediately before the offending instruction."""
    ctr = 0
    for f in nc.m.functions:
        for b in f.blocks:
            insts = b.instructions
            need = any(
                inst.sync_info is not None and len(inst.sync_info.on_wait) > 1
                for inst in insts
            )
            if not need:
                continue
            out = []
            for inst in insts:
                si = inst.sync_info
                if si is not None and len(si.on_wait) > 1:
                    waits = list(si.on_wait)
                    for w in waits[:-1]:
                        nop = mybir.InstNoOp(name=f"I-wsplit-{ctr}", ins=[], outs=[])
                        ctr += 1
                        nop.engine = inst.engine
                        nop.sync_info = _br.SyncInfo(on_wait=[w], on_update=[])
                        out.append(nop)
                    si.on_wait = waits[-1:]
                out.append(inst)
            b.instructions = out
    return ctr


E, TOPK, CAP, H, F, N, NCORES = 8, 2, 512, 2048, 8192, 2048, 8
HT = H // 128                 # 16 hidden tiles
FT = F // 128                 # 64 ffn tiles
TOKC = N // NCORES            # 256 tokens per core
BI = N // 128                 # 16
NSEG = 4
SEGL = 2 * N // NSEG

f32 = mybir.dt.float32
f16 = mybir.dt.float16
bf16 = mybir.dt.bfloat16
i32 = mybir.dt.int32
AOP = mybir.AluOpType
AFT = mybir.ActivationFunctionType
AX = mybir.AxisListType


def build_moe(nc: bass.Bass):
    xtm = nc.dram_tensor("xtm", [2, 128, H], f32, kind="ExternalInput")
    xtt = nc.dram_tensor("xtt", [128, HT * TOKC], bf16, kind="ExternalInput")
    rwT = nc.dram_tensor("rwT", [128, HT * E], f32, kind="ExternalInput")
    w1T = nc.dram_tensor("w1tt", [FT, 128, HT * 128], bf16, kind="ExternalInput")
    w2q = nc.dram_tensor("w2q", [FT, 128, H], bf16, kind="ExternalInput")
    b1t = nc.dram_tensor("b1t", [128, FT], f32, kind="ExternalInput")
    b2r = nc.dram_tensor("b2r", [1, H], f32, kind="ExternalInput")
    cid = nc.dram_tensor("cid", [1, 1], f32, kind="ExternalInput")
    out = nc.dram_tensor("out", [TOKC, H], f32, kind="ExternalOutput")

    rloc = nc.dram_tensor("rloc", [TOKC, 4], f32)
    rall = nc.dram_tensor("rall", [N, 4], f32, addr_space="Shared")
    ebuf8 = nc.dram_tensor("ebuf8", [E, 2 * N], f32)
    e32d = nc.dram_tensor("e32d", [1, 32], f32)
    posd = nc.dram_tensor("posd", [1, 2 * N], f32)
    wd = nc.dram_tensor("wd", [N, 1], f32)

    with tile.TileContext(nc, num_cores=NCORES) as tc:
        with (tc.tile_pool(name="persist", bufs=1) as persist,
              tc.tile_pool(name="s_ps", bufs=2, space="PSUM") as s_ps):
            _body(nc, tc, persist, s_ps, xtm, xtt, rwT, w1T, w2q, b1t, b2r,
                  cid, out, rloc, rall, ebuf8, e32d, posd, wd)
    return nc


def _body(nc, tc, persist, s_ps, xtm, xtt, rwT, w1T, w2q, b1t, b2r, cid, out,
          rloc, rall, ebuf8, e32d, posd, wd):
    RG = [list(range(NCORES))]
    sc = persist

    # ---- persistent tiles ----
    b2b = persist.tile([128, H], f32, tag="b2b")
    cidb = persist.tile([128, 1], f32, tag="cidb")
    b1sb = persist.tile([128, FT], f32, tag="b1sb")
    rws = persist.tile([128, HT * E], f32, tag="rws")
    xcT = persist.tile([128, HT * TOKC], bf16, tag="xcT")
    iotap = persist.tile([128, 1], f32, tag="iotap")
    wt = persist.tile([128, 2], f32, tag="wt")
    widx = persist.tile([128, 2], i32, tag="widx")

    # router inputs first (unblock the tensor queue), then the rest
    nc.gpsimd.dma_start(out=rws[:], in_=rwT[:, :])
    nc.gpsimd.dma_start(out=cidb[:], in_=cid[0:1, :].partition_broadcast(128).opt())
    nc.scalar.dma_start(out=xcT[:], in_=xtt[:, :])
    nc.gpsimd.dma_start(out=b1sb[:], in_=b1t[:, :])
    nc.gpsimd.dma_start(out=b2b[:], in_=b2r[0:1, :].partition_broadcast(128).opt())
    ip = persist.tile([128, 1], i32, tag="ip")
    nc.gpsimd.iota(ip[:], pattern=[[0, 1]], base=0, channel_multiplier=1)
    nc.vector.tensor_copy(out=iotap[:], in_=ip[:])

    # ============ Phase R: sharded router (own 256 tokens, fp32) ============
    with (tc.tile_pool(name="r_x", bufs=2) as r_x,
          tc.tile_pool(name="r_ps", bufs=2, space="PSUM") as r_ps,
          tc.tile_pool(name="r_sb", bufs=2) as r_sb):
        for tt2 in range(2):
            xt_t = r_x.tile([128, H], f32, tag="xt_t")
            for qq in range(2):
                eng = nc.sync if qq == 0 else nc.scalar
                eng.dma_start(
                    out=xt_t[:, qq * (H // 2):(qq + 1) * (H // 2)],
                    in_=xtm[tt2, :, qq * (H // 2):(qq + 1) * (H // 2)])
            ps = r_ps.tile([128, E], f32, tag="r_ps")
            for hc in range(HT):
                nc.tensor.matmul(
                    out=ps[:], lhsT=xt_t[:, hc * 128:(hc + 1) * 128],
                    rhs=rws[:, hc * E:(hc + 1) * E],
                    start=(hc == 0), stop=(hc == HT - 1))
            lsb = r_sb.tile([128, E], f32, tag="lsb")
            nc.vector.tensor_copy(out=lsb[:], in_=ps[:])
            mx = r_sb.tile([128, 1], f32, tag="mx")
            nc.vector.tensor_reduce(out=mx[:], in_=lsb[:], op=AOP.max, axis=AX.X)
            nm = r_sb.tile([128, 1], f32, tag="nm")
            nc.vector.tensor_scalar_mul(nm[:], mx[:], -1.0)
            ex = r_sb.tile([128, E], f32, tag="ex")
            ssum = r_sb.tile([128, 1], f32, tag="ssum")
            nc.scalar.activation(out=ex[:], in_=lsb[:], func=AFT.Exp,
                                 bias=nm[:], scale=1.0, accum_out=ssum[:])
            rcp = r_sb.tile([128, 1], f32, tag="rcp")
            nc.vector.reciprocal(out=rcp[:], in_=ssum[:])
            pr = r_sb.tile([128, E], f32, tag="pr")
            nc.vector.tensor_scalar_mul(pr[:], ex[:], rcp[:])
            mx8 = r_sb.tile([128, 8], f32, tag="mx8")
            ix8 = r_sb.tile([128, 8], mybir.dt.uint32, tag="ix8")
            nc.vector.max_with_indices(out_max=mx8[:], out_indices=ix8[:],
                                       in_=pr[:])
            rv = r_sb.tile([128, 4], f32, tag="rv")
            nc.vector.tensor_copy(out=rv[:, 0:1], in_=ix8[:, 0:1])
            nc.vector.tensor_copy(out=rv[:, 1:2], in_=ix8[:, 1:2])
            nc.vector.tensor_copy(out=rv[:, 2:3], in_=mx8[:, 0:1])
            nc.vector.tensor_copy(out=rv[:, 3:4], in_=mx8[:, 1:2])
            nc.gpsimd.dma_start(out=rloc[tt2 * 128:(tt2 + 1) * 128, :], in_=rv[:])
        nc.gpsimd.collective_compute(
            "AllGather", AOP.bypass,
            replica_groups=RG,
            ins=[rloc[:, :].opt()],
            outs=[rall[:, :].opt()])

    # contiguous per-partition load of the gathered routing, (p, b) layout
    rb = persist.tile([128, 16 * 4], f32, tag="rb")
    nc.gpsimd.dma_start(
        out=rb[:], in_=rall[:, :].rearrange("(p b) c -> p (b c)", p=128))
    rbv = rb[:].rearrange("p (b c) -> p b c", c=4)
    e0a = persist.tile([128, BI], f32, tag="e0a")
    e1a = persist.tile([128, BI], f32, tag="e1a")
    p0a = persist.tile([128, BI], f32, tag="p0a")
    p1a = persist.tile([128, BI], f32, tag="p1a")
    nc.vector.tensor_copy(out=e0a[:], in_=rbv[:, :, 0].opt())
    nc.vector.tensor_copy(out=e1a[:], in_=rbv[:, :, 1].opt())
    nc.vector.tensor_copy(out=p0a[:], in_=rbv[:, :, 2].opt())
    nc.vector.tensor_copy(out=p1a[:], in_=rbv[:, :, 3].opt())

    # ============ Phase S: one-hot + 4-way segmented scan (fp16) ============
    # pack expert ids, roundtrip through DRAM to get the (k, t)-ordered
    # row (t = p*16 + b), then a broadcast load into [32, 1024]: partition
    # (e, seg) scans its 1024-long segment; segment offsets fixed up via a
    # small triangular matmul over the per-segment totals (issued after F1
    # on the tensor queue; all deps are ready long before it reaches PE).
    ip32 = sc.tile([32, 1], i32, tag="ip32")
    ip32f = sc.tile([32, 1], f32, tag="ip32f")
    nc.gpsimd.iota(ip32[:], pattern=[[0, 1]], base=0, channel_multiplier=1)
    nc.vector.tensor_copy(out=ip32f[:], in_=ip32[:])
    eri = sc.tile([1, 32], i32, tag="eri")
    nc.gpsimd.iota(eri[:], pattern=[[1, E], [0, NSEG]], base=0,
                   channel_multiplier=0)
    erf = sc.tile([1, 32], f32, tag="erf")
    nc.vector.tensor_copy(out=erf[:], in_=eri[:])
    nc.gpsimd.dma_start(out=e32d[0:1, :], in_=erf[:])
    eidx = sc.tile([32, 1], f32, tag="eidx")
    nc.gpsimd.dma_start(
        out=eidx[:, :],
        in_=e32d[0:1, :].rearrange("a (c u) -> (a c) u", u=1))
    sidx = sc.tile([32, 1], f32, tag="sidx")
    nc.vector.scalar_tensor_tensor(out=sidx[:], in0=eidx[:],
                                   scalar=-float(NSEG), in1=ip32f[:],
                                   op0=AOP.mult, op1=AOP.add)
    # Mt[p', p] = same expert and seg(p') < seg(p): exclusive prefix mask
    jmi = sc.tile([32, 32], i32, tag="jmi")
    nc.gpsimd.iota(jmi[:], pattern=[[0, E], [1, NSEG]], base=0,
                   channel_multiplier=0)
    jm = sc.tile([32, 32], f32, tag="jm")
    nc.vector.tensor_copy(out=jm[:], in_=jmi[:])
    eci = sc.tile([32, 32], i32, tag="eci")
    nc.gpsimd.iota(eci[:], pattern=[[1, E], [0, NSEG]], base=0,
                   channel_multiplier=0)
    ec = sc.tile([32, 32], f32, tag="ec")
    nc.vector.tensor_copy(out=ec[:], in_=eci[:])
    Mt = sc.tile([32, 32], f16, tag="Mt")
    me32 = sc.tile([32, 32], f32, tag="me32")
    nc.vector.tensor_scalar(out=me32[:], in0=ec[:],
                            scalar1=eidx[:], scalar2=None, op0=AOP.is_equal)
    ms32 = sc.tile([32, 32], f32, tag="ms32")
    nc.vector.tensor_scalar(out=ms32[:], in0=jm[:],
                            scalar1=sidx[:], scalar2=None, op0=AOP.is_gt)
    nc.vector.tensor_tensor(out=Mt[:], in0=me32[:], in1=ms32[:],
                            op=AOP.mult)
    sel4 = sc.tile([32, NSEG], f16, tag="sel4")
    iseg = sc.tile([32, NSEG], i32, tag="iseg")
    nc.gpsimd.iota(iseg[:], pattern=[[1, NSEG]], base=0, channel_multiplier=0)
    isegf = sc.tile([32, NSEG], f32, tag="isegf")
    nc.vector.tensor_copy(out=isegf[:], in_=iseg[:])
    nc.vector.tensor_scalar(out=sel4[:], in0=isegf[:], scalar1=sidx[:],
                            scalar2=None, op0=AOP.is_equal)

    e01 = sc.tile([128, 32], f32, tag="e01")
    nc.vector.tensor_copy(out=e01[:, 0:16], in_=e0a[:])
    nc.vector.tensor_copy(out=e01[:, 16:32], in_=e1a[:])
    for e in range(E):
        nc.gpsimd.dma_start(
            out=ebuf8[e:e + 1, :].rearrange(
                "a (k p b) -> (a p) k b", k=2, p=128),
            in_=e01[:].rearrange("p (k b) -> p k b", k=2))
    ohsrc = sc.tile([32, SEGL], f32, tag="ohsrc")
    nc.gpsimd.dma_start(
        out=ohsrc[:],
        in_=ebuf8[:, :].rearrange("e (s c) -> (e s) c", s=NSEG))
    ohcat = sc.tile([32, SEGL], f16, tag="ohcat")
    nc.vector.tensor_scalar(out=ohcat[:], in0=ohsrc[:], scalar1=eidx[:],
                            scalar2=None, op0=AOP.is_equal)
    ones2n = sc.tile([32, SEGL], f16, tag="ones2n")
    nc.vector.memset(ones2n[:], 1.0)
    cum = sc.tile([32, SEGL], f16, tag="cum")
    nc.vector.tensor_tensor_scan(out=cum[:], data0=ones2n[:], data1=ohcat[:],
                                 initial=0.0, op0=AOP.mult, op1=AOP.add)
    tot32 = sc.tile([32, 1], f16, tag="tot32")
    with nc.allow_low_precision(reason="segment counts <= 1024, f16-exact"):
        nc.vector.tensor_reduce(out=tot32[:], in_=ohcat[:], op=AOP.add,
                                axis=AX.X)

    # ============ Phase F1 (dense, own 256 tokens) ============
    with tc.tile_pool(name="g", bufs=1) as g_pool:
        g = []
        with (tc.tile_pool(name="f1_w", bufs=4) as f1_w,
              tc.tile_pool(name="f1_ps", bufs=2, space="PSUM") as f1_ps):
            for ft in range(FT):
                w1_t = f1_w.tile([128, HT * 128], bf16, tag="w1_t")
                QW = HT * 128 // 2
                for qq in range(2):
                    eng = nc.sync if qq == 0 else nc.scalar
                    eng.dma_start(
                        out=w1_t[:, qq * QW:(qq + 1) * QW],
                        in_=w1T[ft, :, qq * QW:(qq + 1) * QW])
                ps = f1_ps.tile([128, TOKC], f32, tag="f1_ps")
                for hc in range(HT):
                    nc.tensor.matmul(
                        out=ps[:], lhsT=w1_t[:, hc * 128:(hc + 1) * 128],
                        rhs=xcT[:, hc * TOKC:(hc + 1) * TOKC],
                        start=(hc == 0), stop=(hc == HT - 1))
                gt = g_pool.tile([128, TOKC], bf16, tag=f"g_{ft}")
                nc.scalar.activation(out=gt[:], in_=ps[:], func=AFT.Gelu,
                                     bias=b1sb[:, ft:ft + 1], scale=1.0)
                g.append(gt)

        # ---- scan fixup (tensor) + per-token weight chain ----
        offp = s_ps.tile([32, 1], f32, tag="offp")
        nc.tensor.matmul(out=offp[:], lhsT=Mt[:], rhs=tot32[:],
                         start=True, stop=True)
        off32 = sc.tile([32, 1], f32, tag="off32")
        nc.vector.tensor_copy(out=off32[:], in_=offp[:])
        cumf = sc.tile([32, SEGL], f16, tag="cumf")
        nc.vector.tensor_scalar(out=cumf[:], in0=cum[:], scalar1=off32[:],
                                scalar2=None, op0=AOP.add)
        ohcum = sc.tile([32, SEGL], f16, tag="ohcum")
        nc.vector.tensor_tensor(out=ohcum[:], in0=ohcat[:], in1=cumf[:],
                                op=AOP.mult)
        posrow = sc.tile([1, 2 * N], f32, tag="posrow")
        for s in range(NSEG):
            for ch in range(SEGL // 512):
                pps = s_ps.tile([1, 512], f32, tag="pps")
                nc.tensor.matmul(out=pps[:], lhsT=sel4[:, s:s + 1],
                                 rhs=ohcum[:, ch * 512:(ch + 1) * 512],
                                 start=True, stop=True)
                nc.vector.tensor_scalar_add(
                    posrow[:, s * SEGL + ch * 512:s * SEGL + (ch + 1) * 512],
                    pps[:], -1.0)
        nc.gpsimd.dma_start(out=posd[:, 0:N], in_=posrow[:, 0:N])
        nc.gpsimd.dma_start(out=posd[:, N:2 * N], in_=posrow[:, N:2 * N])

        # w(t) = p0*(pos0<CAP) + p1*(pos1<CAP) for all tokens -> wd,
        # then indirect-gather own 256 into wt[128, 2] (col = tok//128)
        pos0a = sc.tile([128, BI], f32, tag="pos0a")
        pos1a = sc.tile([128, BI], f32, tag="pos1a")
        nc.gpsimd.dma_start(
            out=pos0a[:],
            in_=posd[0:1, 0:N].rearrange("a (p b) -> (a p) b", p=128))
        nc.gpsimd.dma_start(
            out=pos1a[:],
            in_=posd[0:1, N:2 * N].rearrange("a (p b) -> (a p) b", p=128))
        wall = sc.tile([128, BI], f32, tag="wall")
        k0 = sc.tile([128, BI], f32, tag="k0")
        nc.vector.tensor_scalar(out=k0[:], in0=pos0a[:], scalar1=float(CAP),
                                scalar2=None, op0=AOP.is_lt)
        nc.vector.tensor_tensor(out=k0[:], in0=k0[:], in1=p0a[:], op=AOP.mult)
        k1 = sc.tile([128, BI], f32, tag="k1")
        nc.vector.tensor_scalar(out=k1[:], in0=pos1a[:], scalar1=float(CAP),
                                scalar2=None, op0=AOP.is_lt)
        nc.vector.tensor_tensor(out=k1[:], in0=k1[:], in1=p1a[:], op=AOP.mult)
        nc.vector.tensor_tensor(out=wall[:], in0=k0[:], in1=k1[:], op=AOP.add)
        nc.gpsimd.dma_start(
            out=wd[:, :].rearrange("(p b) a -> p (b a)", p=128), in_=wall[:])
        wif = sc.tile([128, 2], f32, tag="wif")
        nc.vector.scalar_tensor_tensor(out=wif[:, 0:1], in0=cidb[:],
                                       scalar=float(TOKC), in1=iotap[:],
                                       op0=AOP.mult, op1=AOP.add)
        nc.vector.tensor_scalar_add(wif[:, 1:2], wif[:, 0:1], 128.0)
        nc.vector.tensor_copy(out=widx[:], in_=wif[:])
        for q in range(2):
            nc.gpsimd.indirect_dma_start(
                out=wt[:, q:q + 1], out_offset=None,
                in_=wd[:, :],
                in_offset=bass.IndirectOffsetOnAxis(ap=widx[:, q:q + 1], axis=0))

        # ============ Phase F2 + epilogue, H-chunk-major ============
        with (tc.tile_pool(name="f2_w", bufs=12) as f2_w,
              tc.tile_pool(name="f2_ps", bufs=2, space="PSUM") as f2_ps,
              tc.tile_pool(name="cbp", bufs=4) as cbp):
            for ci in range(4):
                psq = []
                for tt2 in range(2):
                    psq_t = f2_ps.tile([128, 512], f32, tag=f"f2_ps_{tt2}")
                    psq.append(psq_t)
                for fc in range(FT):
                    w2_t = f2_w.tile([128, 512], bf16, tag="w2_t")
                    eng = nc.sync if (fc % 2 == 0) else nc.scalar
                    eng.dma_start(out=w2_t[:],
                                  in_=w2q[fc, :, ci * 512:(ci + 1) * 512])
                    for tt2 in range(2):
                        nc.tensor.matmul(
                            out=psq[tt2][:],
                            lhsT=g[fc][:, tt2 * 128:(tt2 + 1) * 128],
                            rhs=w2_t[:],
                            start=(fc == 0), stop=(fc == FT - 1))
                for tt2 in range(2):
                    hs = cbp.tile([128, 512], f32, tag="hs")
                    nc.vector.tensor_tensor(
                        out=hs[:], in0=psq[tt2][:],
                        in1=b2b[:, ci * 512:(ci + 1) * 512], op=AOP.add)
                    o = cbp.tile([128, 512], f32, tag="o")
                    nc.vector.tensor_scalar_mul(o[:], hs[:], wt[:, tt2:tt2 + 1])
                    eng = nc.gpsimd if tt2 == 0 else nc.scalar
                    eng.dma_start(
                        out=out[tt2 * 128:(tt2 + 1) * 128,
                                ci * 512:(ci + 1) * 512],
                        in_=o[:])


# ======================== host-side glue ========================

_CACHE = {}


def _prep_inputs(hidden_states, router_w, w1, b1, w2, b2):
    x = np.asarray(hidden_states, np.float32).reshape(-1, H)
    xT = np.ascontiguousarray(x.T)
    w1Tm = np.asarray(w1, np.float32).T.astype(ml_dtypes.bfloat16)
    w2Tm = np.asarray(w2, np.float32).T.astype(ml_dtypes.bfloat16)
    w1tt = np.ascontiguousarray(
        w1Tm.reshape(16, 128, 64, 128).transpose(2, 1, 0, 3)).reshape(64, 128, 2048)
    w2qm = np.ascontiguousarray(w2Tm.reshape(64, 128, 2048))
    base = {
        "w1tt": w1tt,
        "w2q": w2qm,
        "rwT": np.ascontiguousarray(
            np.asarray(router_w, np.float32).T.reshape(16, 128, 8)
            .transpose(1, 0, 2).reshape(128, 128)),
        "b1t": np.ascontiguousarray(np.asarray(b1, np.float32).reshape(FT, 128).T),
        "b2r": np.asarray(b2, np.float32).reshape(1, H),
    }
    xtmf = np.ascontiguousarray(
        xT.reshape(16, 128, 16, 128).transpose(2, 1, 0, 3)).reshape(16, 128, 2048)
    xTb = xT.astype(ml_dtypes.bfloat16)
    ins = []
    for c in range(NCORES):
        m = dict(base)
        m["xtm"] = np.ascontiguousarray(xtmf[2 * c:2 * c + 2])
        # xtt[p, ht*256 + t] = x[c*256 + t, ht*128 + p]
        m["xtt"] = np.ascontiguousarray(
            xTb[:, c * TOKC:(c + 1) * TOKC].reshape(HT, 128, TOKC)
            .transpose(1, 0, 2).reshape(128, HT * TOKC))
        m["cid"] = np.full((1, 1), float(c), np.float32)
        ins.append(m)
    return ins


def _get_nc():
    if "nc" not in _CACHE:
        apply()  # tile drain patch
        nc = bass.Bass(num_devices=NCORES)
        build_moe(nc)
        split_multi_waits(nc)
        _CACHE["nc"] = nc
    return _CACHE["nc"]


def kernel(hidden_states, router_w, w1, b1, w2, b2):
    from concourse.bass_utils import run_bass_kernel_spmd

    orig_shape = np.asarray(hidden_states).shape
    nc = _get_nc()
    ins = _prep_inputs(hidden_states, router_w, w1, b1, w2, b2)
    res = run_bass_kernel_spmd(nc, ins, core_ids=list(range(NCORES)))
    full = np.concatenate([res.results[c]["out"] for c in range(NCORES)], axis=0)
    return full.reshape(orig_shape).astype(np.float32)
